# revision 1
# baseline (speedup 1.0000x reference)
"""GQA attention (B=2, S=2048, D=2048, Hq=16, Hkv=4, hd=128) on 8 TRN2 cores.

Sharding: core c = b*4 + kv handles batch b and kv-head kv (with its 4 query
heads). Each core computes its partial output (A_heads @ Wo_slice); the host
sums the 4 partials per batch and adds the bias.

Per-core kernel (all matmuls fp32r, N>=256 so the PE runs at full rate):
  phase 1: Q^T/K^T per head ([hd, s] layout), V^T -> PE-transpose -> V [s, hd]
  phase 2: S^T = K^T.T-mm [j, i] blocks; exp on ACT (scale folded in; softmax
           max-subtraction skipped, |S*scale| < ~6 so exp is safe in fp32);
           PV accumulates O^T = sum_j V[j,:].T-mm exp tiles; denominators via
           DVE adds + gpsimd partition_all_reduce; normalize deferred to the
           PSUM->SBUF copy of O^T.
  phase 3: out = O^T.T-mm Wo blocks, streamed to DRAM.
"""
import sys

sys.path.insert(0, "/opt/trn_rl_repo")
import numpy as np

B, S, D = 2, 2048, 2048
HQ, HKV, HD = 16, 4, 128
G = HQ // HKV
SCALE = HD ** -0.5
P = 128
NB = 512
DC = D // P     # 16 contraction chunks
SB = S // NB    # 4 seq blocks of 512
ST = S // P     # 16 seq tiles of 128

_CACHE = {}


def _build(reps=(1, 1, 1)):
    from contextlib import ExitStack, nullcontext

    import concourse.bacc as bacc
    import concourse.bass_isa as bass_isa
    import concourse.mybir as mybir
    import concourse.tile as tile
    from concourse.masks import make_identity

    F32 = mybir.dt.float32
    F32R = mybir.dt.float32r
    Exp = mybir.ActivationFunctionType.Exp

    nc = bacc.Bacc("TRN2", target_bir_lowering=False, debug=False)
    xT = nc.dram_tensor("xT", [D, S], F32R, kind="ExternalInput").ap()
    wq = nc.dram_tensor("wq", [D, G * HD], F32R, kind="ExternalInput").ap()
    wk = nc.dram_tensor("wk", [D, HD], F32R, kind="ExternalInput").ap()
    wv = nc.dram_tensor("wv", [D, HD], F32R, kind="ExternalInput").ap()
    wo = nc.dram_tensor("wo", [G * HD, D], F32R, kind="ExternalInput").ap()
    out = nc.dram_tensor("out", [S, D], F32, kind="ExternalOutput").ap()

    r1, r2, _ = reps

    with tile.TileContext(nc) as tc, ExitStack() as stk:
        persist = stk.enter_context(tc.tile_pool(name="persist", bufs=1))
        kt_sb = persist.tile([P, S], F32R)
        v_sb = persist.tile([P, ST, HD], F32R)

        def _loop(r):
            return tc.For_i(0, r, 1) if r > 1 else nullcontext()

        # ---- pass A: K^T and V for the whole sequence ----
        with ExitStack() as pas:
            a1 = pas.enter_context(tc.tile_pool(name="a1", bufs=1))
            xta_pool = pas.enter_context(tc.tile_pool(name="xta", bufs=2))
            vt_pool = pas.enter_context(tc.tile_pool(name="vt", bufs=2))
            ps_kv = pas.enter_context(tc.tile_pool(name="ps_kv", bufs=2, space="PSUM"))
            ps_t = pas.enter_context(tc.tile_pool(name="ps_t", bufs=2, space="PSUM"))

            wk_sb = a1.tile([P, DC, HD], F32R)
            wv_sb = a1.tile([P, DC, HD], F32R)
            ident = a1.tile([P, P], F32)
            make_identity(nc, ident)
            for c in range(DC):
                nc.sync.dma_start(out=wk_sb[:, c, :], in_=wk[c * P:(c + 1) * P, :])
                nc.sync.dma_start(out=wv_sb[:, c, :], in_=wv[c * P:(c + 1) * P, :])

            with _loop(r1):
              for xb in range(SB):
                cols = slice(xb * NB, (xb + 1) * NB)
                xt = xta_pool.tile([P, DC, NB], F32R, name="xt")
                for c in range(DC):
                    nc.sync.dma_start(out=xt[:, c, :], in_=xT[c * P:(c + 1) * P, cols])
                pk = ps_kv.tile([P, NB], F32, name="pk")
                for c in range(DC):
                    nc.tensor.matmul(pk, wk_sb[:, c, :], xt[:, c, :],
                                     start=(c == 0), stop=(c == DC - 1))
                nc.vector.tensor_copy(out=kt_sb[:, cols], in_=pk)
                pv = ps_kv.tile([P, NB], F32, name="pv")
                for c in range(DC):
                    nc.tensor.matmul(pv, wv_sb[:, c, :], xt[:, c, :],
                                     start=(c == 0), stop=(c == DC - 1))
                vt = vt_pool.tile([P, NB], F32, name="vt")
                nc.vector.tensor_copy(out=vt, in_=pv)
                for k in range(NB // P):
                    pt = ps_t.tile([P, P], F32, name="pt")
                    nc.tensor.transpose(pt, vt[:, k * P:(k + 1) * P], ident)
                    nc.vector.tensor_copy(out=v_sb[:, xb * (NB // P) + k, :], in_=pt)

        # ---- pass B: per column block: Q^T, attention, output projection ----
        with ExitStack() as pbs:
            b1 = pbs.enter_context(tc.tile_pool(name="b1", bufs=1))
            wq_sb = b1.tile([P, DC, G * HD], F32R)
            wo_sb = b1.tile([P, G, D], F32R)
            for c in range(DC):
                nc.sync.dma_start(out=wq_sb[:, c, :], in_=wq[c * P:(c + 1) * P, :])
            for h in range(G):
                nc.sync.dma_start(out=wo_sb[:, h, :], in_=wo[h * P:(h + 1) * P, :])
            xtb_pool = pbs.enter_context(tc.tile_pool(name="xtb", bufs=1))
            qt_pool = pbs.enter_context(tc.tile_pool(name="qt", bufs=2))
            ot_pool = pbs.enter_context(tc.tile_pool(name="ot", bufs=1))
            ex_pool = pbs.enter_context(tc.tile_pool(name="ex", bufs=2))
            dn_pool = pbs.enter_context(tc.tile_pool(name="dn", bufs=1))
            st_pool = pbs.enter_context(tc.tile_pool(name="st", bufs=2))
            ps_q = pbs.enter_context(tc.tile_pool(name="ps_q", bufs=1, space="PSUM"))
            ps_s = pbs.enter_context(tc.tile_pool(name="ps_s", bufs=4, space="PSUM"))
            ps_o = pbs.enter_context(tc.tile_pool(name="ps_o", bufs=2, space="PSUM"))
            ps_p = pbs.enter_context(tc.tile_pool(name="ps_p", bufs=1, space="PSUM"))

            with _loop(r2):
              for ib in range(SB):
                icols = slice(ib * NB, (ib + 1) * NB)
                xt = xtb_pool.tile([P, DC, NB], F32R, name="xtb")
                for c in range(DC):
                    nc.sync.dma_start(out=xt[:, c, :], in_=xT[c * P:(c + 1) * P, icols])
                qt_ib = qt_pool.tile([P, G, NB], F32R, name="qt")
                for h in range(G):
                    pq = ps_q.tile([P, NB], F32, name="pq")
                    for c in range(DC):
                        nc.tensor.matmul(pq, wq_sb[:, c, h * HD:(h + 1) * HD],
                                         xt[:, c, :], start=(c == 0), stop=(c == DC - 1))
                    nc.vector.tensor_copy(out=qt_ib[:, h, :], in_=pq)
                ot_ib = ot_pool.tile([P, G, NB], F32R, name="ot")
                for h in range(G):
                    po = ps_o.tile([P, NB], F32, name="po")
                    # exp tiles land in two contiguous 8-tile groups; the
                    # denominator is a wide tree per group after its exps so
                    # no per-j DVE op paces the ST->exp->PV pipeline
                    exbs = [None, None]
                    dgs = [None, None]
                    DEPTH = 3
                    for j in range(ST + DEPTH):
                        if j < ST:
                            g, jo = divmod(j, 8)
                            if jo == 0:
                                exbs[g] = ex_pool.tile([P, 8, NB], F32R, name="ex")
                            pss = ps_s.tile([P, NB], F32, name="pss")
                            nc.tensor.matmul(pss, kt_sb[:, j * P:(j + 1) * P],
                                             qt_ib[:, h, :], start=True, stop=True)
                            nc.scalar.activation(out=exbs[g][:, jo, :], in_=pss,
                                                 func=Exp, scale=SCALE)
                            if jo == 7:
                                exf = exbs[g].bitcast(F32)
                                t4 = dn_pool.tile([P, 4, NB], F32, name=f"t4{g}")
                                nc.vector.tensor_add(out=t4, in0=exf[:, 0:4, :],
                                                     in1=exf[:, 4:8, :])
                                nc.vector.tensor_add(out=t4[:, 0:2, :], in0=t4[:, 0:2, :],
                                                     in1=t4[:, 2:4, :])
                                nc.vector.tensor_add(out=t4[:, 0, :], in0=t4[:, 0, :],
                                                     in1=t4[:, 1, :])
                                dgs[g] = t4
                        jc = j - DEPTH
                        if jc >= 0:
                            nc.tensor.matmul(po, v_sb[:, jc, :], exbs[jc // 8][:, jc % 8, :],
                                             start=(jc == 0), stop=(jc == ST - 1))
                    dsum = dn_pool.tile([P, NB], F32, name="dsum")
                    nc.vector.tensor_add(out=dsum, in0=dgs[0][:, 0, :], in1=dgs[1][:, 0, :])
                    dred = dn_pool.tile([P, NB], F32, name="dred")
                    nc.gpsimd.partition_all_reduce(dred, dsum, P, bass_isa.ReduceOp.add)
                    nc.vector.reciprocal(out=dsum, in_=dred)
                    nc.vector.tensor_tensor(out=ot_ib[:, h, :], in0=po,
                                            in1=dsum, op=mybir.AluOpType.mult)
                for t in range(4):
                    it = 4 * ib + t
                    for nb in range(D // NB):
                        pso = ps_p.tile([P, NB], F32, name="pso")
                        for h in range(G):
                            nc.tensor.matmul(pso, ot_ib[:, h, t * P:(t + 1) * P],
                                             wo_sb[:, h, nb * NB:(nb + 1) * NB],
                                             start=(h == 0), stop=(h == G - 1))
                        so = st_pool.tile([P, NB], F32, name="so")
                        nc.vector.tensor_copy(out=so, in_=pso)
                        nc.sync.dma_start(out=out[it * P:(it + 1) * P, nb * NB:(nb + 1) * NB],
                                          in_=so)

    nc.compile()
    return nc


def _get_nc():
    if "nc" not in _CACHE:
        _CACHE["nc"] = _build()
    return _CACHE["nc"]


def timed_runner(reps):
    nc = _build(reps)
    return make_runner(nc)


def make_runner(nc, n_cores=8):
    """Persistent jitted SPMD runner (mirrors bass2jax.run_bass_via_pjrt's
    multi-core path, without donation so the executable can be re-invoked on
    device-resident inputs for timing)."""
    import jax
    from jax.experimental.shard_map import shard_map
    from jax.sharding import Mesh, PartitionSpec

    import concourse.mybir as mybir
    from concourse import bass2jax

    bass2jax.install_neuronx_cc_hook()
    partition_name = nc.partition_id_tensor.name if nc.partition_id_tensor else None
    in_names, out_names, out_avals, zero_shapes = [], [], [], []
    for alloc in nc.m.functions[0].allocations:
        if not isinstance(alloc, mybir.MemoryLocationSet):
            continue
        name = alloc.memorylocations[0].name
        if alloc.kind == "ExternalInput":
            if name != partition_name:
                in_names.append(name)
        elif alloc.kind == "ExternalOutput":
            out_names.append(name)
            shape = tuple(alloc.tensor_shape)
            dtype = mybir.dt.np(alloc.dtype)
            out_avals.append(jax.core.ShapedArray(shape, dtype))
            zero_shapes.append((shape, dtype))
    n_params = len(in_names)
    all_in_names = tuple(in_names + out_names)
    if partition_name is not None:
        all_in_names = all_in_names + (partition_name,)

    def _body(*args):
        operands = list(args)
        if partition_name is not None:
            operands.append(bass2jax.partition_id_tensor())
        outs = bass2jax._bass_exec_p.bind(
            *operands,
            out_avals=tuple(out_avals),
            in_names=all_in_names,
            out_names=tuple(out_names),
            lowering_input_output_aliases=(),
            sim_require_finite=True,
            sim_require_nnan=True,
            nc=nc,
        )
        return tuple(outs)

    devices = jax.devices()[:n_cores]
    mesh = Mesh(np.asarray(devices), ("core",))
    n_outs = len(out_names)
    fn = jax.jit(
        shard_map(_body, mesh=mesh,
                  in_specs=(PartitionSpec("core"),) * (n_params + n_outs),
                  out_specs=(PartitionSpec("core"),) * n_outs,
                  check_rep=False),
        keep_unused=True,
    )
    return fn, in_names, out_names, zero_shapes, mesh


def _get_runner():
    if "runner" not in _CACHE:
        _CACHE["runner"] = make_runner(_get_nc())
    return _CACHE["runner"]


def run_cores(in_maps):
    """Run the 8-core SPMD program; returns list of per-core {name: array}."""
    import jax

    fn, in_names, out_names, zero_shapes, mesh = _get_runner()
    n = len(in_maps)
    concat_in = [np.concatenate([np.asarray(in_maps[c][nm]) for c in range(n)], axis=0)
                 for nm in in_names]
    concat_zero = [np.zeros((n * s[0], *s[1:]), dt) for s, dt in zero_shapes]
    outs = fn(*concat_in, *concat_zero)
    outs = [np.asarray(o) for o in outs]
    return [
        {nm: outs[i].reshape(n, *zero_shapes[i][0])[c] for i, nm in enumerate(out_names)}
        for c in range(n)
    ]


def shard_inputs(x, Wq, Wk, Wv, Wo):
    in_maps = []
    for b in range(B):
        xTb = np.ascontiguousarray(x[b].T)
        for kv in range(HKV):
            in_maps.append({
                "xT": xTb,
                "wq": np.ascontiguousarray(Wq[:, kv * G * HD:(kv + 1) * G * HD]),
                "wk": np.ascontiguousarray(Wk[:, kv * HD:(kv + 1) * HD]),
                "wv": np.ascontiguousarray(Wv[:, kv * HD:(kv + 1) * HD]),
                "wo": np.ascontiguousarray(Wo[kv * G * HD:(kv + 1) * G * HD, :]),
            })
    return in_maps


def kernel(x, Wq, Wk, Wv, Wo, bo):
    x = np.asarray(x, np.float32)
    Wq = np.asarray(Wq, np.float32)
    Wk = np.asarray(Wk, np.float32)
    Wv = np.asarray(Wv, np.float32)
    Wo = np.asarray(Wo, np.float32)
    bo = np.asarray(bo, np.float32)
    results = run_cores(shard_inputs(x, Wq, Wk, Wv, Wo))
    out = np.empty((B, S, D), np.float32)
    for b in range(B):
        out[b] = results[4 * b]["out"]
        for kv in range(1, HKV):
            out[b] += results[4 * b + kv]["out"]
        out[b] += bo
    return out



# revision 2
# speedup vs baseline: 1.2247x; 1.2247x over previous
"""GQA attention (B=2, S=2048, D=2048, Hq=16, Hkv=4, hd=128) on 8 TRN2 cores, v2.

Sharding: core c = b*4 + kv handles batch b and kv-head kv (with its 4 query
heads). Each core computes its partial output (A_heads @ Wo_slice); the host
sums the 4 partials per batch and adds the bias.

v2 vs v1: everything in bf16 (PE rate is the same 1 row/cycle as fp32r, but
DMA/SBUF halve and DVE gets 2x/4x 16-bit modes), x is loaded once (Q^T for the
whole sequence is kept in SBUF), and the PE instruction stream is packed so the
tensor engine never waits on the activation engine:
  pass A: Q^T/K^T for all blocks ([hd, s] layout), V^T -> PE-transpose -> V.
          Q for the last block is deferred into pass B as PE filler work.
  pass B: per 512-column block ib, per head: 16 j-steps of [ST matmul, paired
          1024-wide exp on ACT, PV matmul]; one filler matmul per step keeps
          PE ahead of ACT (fillers: deferred Q-proj for ib==0, out-projection
          of block ib-1 otherwise). Softmax denominators: bf16 tree adds split
          across DVE/Pool, partition_all_reduce on Pool, reciprocal+normalize
          on DVE -- nothing in the PE stream stalls on them.
"""
import sys

sys.path.insert(0, "/opt/trn_rl_repo")
import numpy as np

B, S, D = 2, 2048, 2048
HQ, HKV, HD = 16, 4, 128
G = HQ // HKV
SCALE = HD ** -0.5
P = 128
NB = 512
DC = D // P     # 16 contraction chunks
SB = S // NB    # 4 seq blocks of 512
ST = S // P     # 16 seq tiles of 128
DEPTH = 4       # PV trails ST by this many j-steps

_CACHE = {}


def _build(reps=(1, 1, 1)):
    from contextlib import ExitStack, nullcontext

    import concourse.bacc as bacc
    import concourse.bass_isa as bass_isa
    import concourse.mybir as mybir
    import concourse.tile as tile
    from concourse.masks import make_identity

    F32 = mybir.dt.float32
    BF16 = mybir.dt.bfloat16
    Exp = mybir.ActivationFunctionType.Exp
    MULT = mybir.AluOpType.mult

    nc = bacc.Bacc("TRN2", target_bir_lowering=False, debug=False)
    xT = nc.dram_tensor("xT", [D, S], BF16, kind="ExternalInput").ap()
    wq = nc.dram_tensor("wq", [D, G * HD], BF16, kind="ExternalInput").ap()
    wk = nc.dram_tensor("wk", [D, HD], BF16, kind="ExternalInput").ap()
    wv = nc.dram_tensor("wv", [D, HD], BF16, kind="ExternalInput").ap()
    wo = nc.dram_tensor("wo", [G * HD, D], BF16, kind="ExternalInput").ap()
    out = nc.dram_tensor("out", [S, D], F32, kind="ExternalOutput").ap()

    r1, r2, _ = reps

    with tile.TileContext(nc) as tc, ExitStack() as stk:
        persist = stk.enter_context(tc.tile_pool(name="persist", bufs=1))
        kt_sb = persist.tile([P, S], BF16)          # K^T [hd, j]
        v_sb = persist.tile([P, ST, HD], BF16)      # V   [j, (jt, hd)]
        qt_sb = persist.tile([P, G, S], BF16)       # Q^T [hd, (h, i)]
        xt3 = persist.tile([P, DC, NB], BF16)       # x^T cols of last block
        wq_sb = persist.tile([P, DC, G * HD], BF16)
        wk_sb = persist.tile([P, DC, HD], BF16)
        wv_sb = persist.tile([P, DC, HD], BF16)
        wo_sb = persist.tile([P, G, D], BF16)
        ident = persist.tile([P, P], BF16)
        ones = persist.tile([P, P], BF16)
        make_identity(nc, ident)
        nc.gpsimd.memset(ones, 1.0)
        for c in range(DC):
            nc.sync.dma_start(out=wq_sb[:, c, :], in_=wq[c * P:(c + 1) * P, :])
            nc.sync.dma_start(out=wk_sb[:, c, :], in_=wk[c * P:(c + 1) * P, :])
            nc.sync.dma_start(out=wv_sb[:, c, :], in_=wv[c * P:(c + 1) * P, :])
        for h in range(G):
            nc.sync.dma_start(out=wo_sb[:, h, :], in_=wo[h * P:(h + 1) * P, :])

        def _loop(r):
            return tc.For_i(0, r, 1) if r > 1 else nullcontext()

        # ---- pass A: K^T, V for all blocks; Q^T for blocks 0..SB-2 ----
        with ExitStack() as pas:
            xta_pool = pas.enter_context(tc.tile_pool(name="xta", bufs=2))
            vt_pool = pas.enter_context(tc.tile_pool(name="vt", bufs=2))
            ps_a = pas.enter_context(tc.tile_pool(name="ps_a", bufs=2, space="PSUM"))

            with _loop(r1):
              for xb in range(SB):
                cols = slice(xb * NB, (xb + 1) * NB)
                xt = xt3 if xb == SB - 1 else xta_pool.tile([P, DC, NB], BF16, name="xt")
                for c in range(DC):
                    nc.sync.dma_start(out=xt[:, c, :], in_=xT[c * P:(c + 1) * P, cols])
                pk = ps_a.tile([P, NB], F32, name="pkv")
                for c in range(DC):
                    nc.tensor.matmul(pk, wk_sb[:, c, :], xt[:, c, :],
                                     start=(c == 0), stop=(c == DC - 1))
                nc.vector.tensor_copy(out=kt_sb[:, cols], in_=pk)
                pv = ps_a.tile([P, NB], F32, name="pkv")
                for c in range(DC):
                    nc.tensor.matmul(pv, wv_sb[:, c, :], xt[:, c, :],
                                     start=(c == 0), stop=(c == DC - 1))
                vt = vt_pool.tile([P, NB], BF16, name="vt")
                nc.vector.tensor_copy(out=vt, in_=pv)
                # Q chains keep PE busy while the vt copy completes
                if xb != SB - 1:
                    for h in range(G):
                        pq = ps_a.tile([P, NB], F32, name="pq")
                        for c in range(DC):
                            nc.tensor.matmul(pq, wq_sb[:, c, h * HD:(h + 1) * HD],
                                             xt[:, c, :], start=(c == 0), stop=(c == DC - 1))
                        nc.vector.tensor_copy(out=qt_sb[:, h, cols], in_=pq)
                for k in range(NB // P):
                    pt = ps_a.tile([P, P], BF16, name="pt")
                    nc.tensor.transpose(pt, vt[:, k * P:(k + 1) * P], ident)
                    nc.vector.tensor_copy(out=v_sb[:, xb * (NB // P) + k, :], in_=pt)

        # ---- pass B: attention + out-projection, PE stream fully packed ----
        with ExitStack() as pbs:
            ex_pool = pbs.enter_context(tc.tile_pool(name="ex", bufs=2))
            ot_pool = pbs.enter_context(tc.tile_pool(name="otp", bufs=2))
            dn_pool = pbs.enter_context(tc.tile_pool(name="dn", bufs=2))
            st_pool = pbs.enter_context(tc.tile_pool(name="st", bufs=4))
            ps_st = pbs.enter_context(tc.tile_pool(name="ps_st", bufs=2, space="PSUM"))
            ps_po = pbs.enter_context(tc.tile_pool(name="ps_po", bufs=1, space="PSUM"))
            ps_aux = pbs.enter_context(tc.tile_pool(name="ps_aux", bufs=2, space="PSUM"))
            ps_pd = pbs.enter_context(tc.tile_pool(name="ps_pd", bufs=1, space="PSUM"))

            with _loop(r2):
                ot_tiles = {}
                aux_state = {}

                def q3_fillers():
                    # deferred Q projection for the last block: 64 matmuls
                    fillers = []
                    icols3 = slice((SB - 1) * NB, SB * NB)
                    for h in range(G):
                        for c in range(DC):
                            def go(h=h, c=c):
                                if c == 0:
                                    aux_state["q"] = ps_aux.tile([P, NB], F32, name="aux")
                                pq = aux_state["q"]
                                nc.tensor.matmul(pq, wq_sb[:, c, h * HD:(h + 1) * HD],
                                                 xt3[:, c, :], start=(c == 0), stop=(c == DC - 1))
                                if c == DC - 1:
                                    nc.vector.tensor_copy(out=qt_sb[:, h, icols3], in_=pq)
                            fillers.append(go)
                    return fillers

                def outproj_fillers(ib):
                    # 16 (t, nb) groups x 4 h-matmuls; two groups in flight so
                    # each group's h3 (which waits on the newest normalize) sits
                    # late in the queue
                    fillers = []
                    ot = ot_tiles[ib]
                    groups = [(t, nb) for t in range(4) for nb in range(D // NB)]
                    for p0 in range(0, 16, 2):
                        pair = groups[p0:p0 + 2]
                        for h in range(G):
                            for t, nb in pair:
                                def go(t=t, nb=nb, h=h, ib=ib, ot=ot):
                                    key = ("o", t, nb)
                                    if h == 0:
                                        aux_state[key] = ps_aux.tile([P, NB], F32, name="aux")
                                    pso = aux_state[key]
                                    nc.tensor.matmul(pso, ot[:, h, t * P:(t + 1) * P],
                                                     wo_sb[:, h, nb * NB:(nb + 1) * NB],
                                                     start=(h == 0), stop=(h == G - 1))
                                    if h == G - 1:
                                        so = st_pool.tile([P, NB], F32, name="so")
                                        # drain on DVE only: ACT must stay a
                                        # pure exp pipeline, and gpsimd cannot
                                        # read PSUM
                                        nc.vector.tensor_copy(out=so, in_=pso)
                                        it = 4 * ib + t
                                        nc.sync.dma_start(
                                            out=out[it * P:(it + 1) * P, nb * NB:(nb + 1) * NB],
                                            in_=so)
                                fillers.append(go)
                    return fillers

                pending_tail = [None]

                def head_tail(ib, h, dsum, oun):
                    # denominator broadcast-sum via PE ones-matmul (replicates
                    # the partition sum to all 128 partitions in fp32), then
                    # reciprocal on DVE + normalize on Pool (all-SBUF).
                    # Scheduled at step 8 of the NEXT head's j-loop so the PE
                    # never waits on the add tree.
                    def go():
                        pd = ps_pd.tile([P, NB], F32, name="pd")
                        nc.tensor.matmul(pd, ones, dsum, start=True, stop=True)
                        rec = dn_pool.tile([P, NB], F32, name="rec")
                        nc.vector.reciprocal(out=rec, in_=pd)
                        nc.gpsimd.tensor_tensor(out=ot_tiles[ib][:, h, :], in0=oun,
                                                in1=rec, op=MULT)
                    return go

                for ib in range(SB):
                    icols = slice(ib * NB, (ib + 1) * NB)
                    ot_tiles[ib] = ot_pool.tile([P, G, NB], BF16, name="ot")
                    if ib == 0:
                        fillers, delay = q3_fillers(), 0
                    else:
                        fillers, delay = outproj_fillers(ib - 1), 4
                    slot = 0
                    for h in range(G):
                        ex = ex_pool.tile([P, ST, NB], BF16, name="ex")
                        po = ps_po.tile([P, NB], F32, name="po")
                        pss = None
                        for j in range(ST + DEPTH):
                            if j < ST:
                                if j % 2 == 0:
                                    pss = ps_st.tile([P, 2, NB], F32, name="pss")
                                nc.tensor.matmul(pss[:, j % 2, :], kt_sb[:, j * P:(j + 1) * P],
                                                 qt_sb[:, h, icols], start=True, stop=True)
                                if j % 2 == 1:
                                    nc.scalar.activation(out=ex[:, j - 1:j + 1, :], in_=pss,
                                                         func=Exp, scale=SCALE)
                            jc = j - DEPTH
                            if jc >= 0:
                                nc.tensor.matmul(po, v_sb[:, jc, :], ex[:, jc, :],
                                                 start=(jc == 0), stop=(jc == ST - 1))
                            if j == 8 and pending_tail[0] is not None:
                                pending_tail[0]()
                                pending_tail[0] = None
                            if j < ST:
                                if slot >= delay and fillers:
                                    fillers.pop(0)()
                                slot += 1
                        # drain the PV accumulator unnormalized right away so
                        # its PSUM bank frees before the next head's PV chain
                        oun = dn_pool.tile([P, NB], BF16, name="oun")
                        nc.vector.tensor_copy(out=oun, in_=po)
                        # softmax denominator add tree: bf16, DVE half (can
                        # start mid-loop) + Pool half; final sums in bf16 feed
                        # the ones-matmul
                        t4a = dn_pool.tile([P, 4, NB], BF16, name="t4a")
                        nc.vector.tensor_add(out=t4a, in0=ex[:, 0:4, :], in1=ex[:, 4:8, :])
                        nc.vector.tensor_add(out=t4a[:, 0:2, :], in0=t4a[:, 0:2, :],
                                             in1=t4a[:, 2:4, :])
                        t4b = dn_pool.tile([P, 4, NB], BF16, name="t4b")
                        nc.gpsimd.tensor_add(out=t4b, in0=ex[:, 8:12, :], in1=ex[:, 12:16, :])
                        nc.gpsimd.tensor_add(out=t4b[:, 0:2, :], in0=t4b[:, 0:2, :],
                                             in1=t4b[:, 2:4, :])
                        dsa = dn_pool.tile([P, NB], BF16, name="dsa")
                        nc.vector.tensor_add(out=dsa, in0=t4a[:, 0, :], in1=t4a[:, 1, :])
                        dsum = dn_pool.tile([P, NB], BF16, name="dsum")
                        nc.gpsimd.tensor_add(out=dsum, in0=t4b[:, 0, :], in1=t4b[:, 1, :])
                        nc.gpsimd.tensor_add(out=dsum, in0=dsum, in1=dsa)
                        pending_tail[0] = head_tail(ib, h, dsum, oun)
                    while fillers:
                        fillers.pop(0)()
                # trailing: last head's tail, then out-projection for block 3
                pending_tail[0]()
                pending_tail[0] = None
                tail = outproj_fillers(SB - 1)
                while tail:
                    tail.pop(0)()

    nc.compile()
    return nc


def _get_nc():
    if "nc" not in _CACHE:
        _CACHE["nc"] = _build()
    return _CACHE["nc"]


def timed_runner(reps):
    nc = _build(reps)
    return make_runner(nc)


def make_runner(nc, n_cores=8):
    """Persistent jitted SPMD runner (mirrors bass2jax.run_bass_via_pjrt's
    multi-core path, without donation so the executable can be re-invoked on
    device-resident inputs for timing)."""
    import jax
    from jax.experimental.shard_map import shard_map
    from jax.sharding import Mesh, PartitionSpec

    import concourse.mybir as mybir
    from concourse import bass2jax

    bass2jax.install_neuronx_cc_hook()
    partition_name = nc.partition_id_tensor.name if nc.partition_id_tensor else None
    in_names, out_names, out_avals, zero_shapes = [], [], [], []
    for alloc in nc.m.functions[0].allocations:
        if not isinstance(alloc, mybir.MemoryLocationSet):
            continue
        name = alloc.memorylocations[0].name
        if alloc.kind == "ExternalInput":
            if name != partition_name:
                in_names.append(name)
        elif alloc.kind == "ExternalOutput":
            out_names.append(name)
            shape = tuple(alloc.tensor_shape)
            dtype = mybir.dt.np(alloc.dtype)
            out_avals.append(jax.core.ShapedArray(shape, dtype))
            zero_shapes.append((shape, dtype))
    n_params = len(in_names)
    all_in_names = tuple(in_names + out_names)
    if partition_name is not None:
        all_in_names = all_in_names + (partition_name,)

    def _body(*args):
        operands = list(args)
        if partition_name is not None:
            operands.append(bass2jax.partition_id_tensor())
        outs = bass2jax._bass_exec_p.bind(
            *operands,
            out_avals=tuple(out_avals),
            in_names=all_in_names,
            out_names=tuple(out_names),
            lowering_input_output_aliases=(),
            sim_require_finite=True,
            sim_require_nnan=True,
            nc=nc,
        )
        return tuple(outs)

    devices = jax.devices()[:n_cores]
    mesh = Mesh(np.asarray(devices), ("core",))
    n_outs = len(out_names)
    fn = jax.jit(
        shard_map(_body, mesh=mesh,
                  in_specs=(PartitionSpec("core"),) * (n_params + n_outs),
                  out_specs=(PartitionSpec("core"),) * n_outs,
                  check_rep=False),
        keep_unused=True,
    )
    return fn, in_names, out_names, zero_shapes, mesh


def _get_runner():
    if "runner" not in _CACHE:
        _CACHE["runner"] = make_runner(_get_nc())
    return _CACHE["runner"]


def run_cores(in_maps):
    """Run the 8-core SPMD program; returns list of per-core {name: array}."""
    import jax

    fn, in_names, out_names, zero_shapes, mesh = _get_runner()
    n = len(in_maps)
    concat_in = [np.concatenate([np.asarray(in_maps[c][nm]) for c in range(n)], axis=0)
                 for nm in in_names]
    concat_zero = [np.zeros((n * s[0], *s[1:]), dt) for s, dt in zero_shapes]
    outs = fn(*concat_in, *concat_zero)
    outs = [np.asarray(o) for o in outs]
    return [
        {nm: outs[i].reshape(n, *zero_shapes[i][0])[c] for i, nm in enumerate(out_names)}
        for c in range(n)
    ]


def shard_inputs(x, Wq, Wk, Wv, Wo):
    import ml_dtypes

    bf16 = ml_dtypes.bfloat16
    in_maps = []
    for b in range(B):
        xTb = np.ascontiguousarray(x[b].T).astype(bf16)
        for kv in range(HKV):
            in_maps.append({
                "xT": xTb,
                "wq": np.ascontiguousarray(Wq[:, kv * G * HD:(kv + 1) * G * HD]).astype(bf16),
                "wk": np.ascontiguousarray(Wk[:, kv * HD:(kv + 1) * HD]).astype(bf16),
                "wv": np.ascontiguousarray(Wv[:, kv * HD:(kv + 1) * HD]).astype(bf16),
                "wo": np.ascontiguousarray(Wo[kv * G * HD:(kv + 1) * G * HD, :]).astype(bf16),
            })
    return in_maps


def kernel(x, Wq, Wk, Wv, Wo, bo):
    x = np.asarray(x, np.float32)
    Wq = np.asarray(Wq, np.float32)
    Wk = np.asarray(Wk, np.float32)
    Wv = np.asarray(Wv, np.float32)
    Wo = np.asarray(Wo, np.float32)
    bo = np.asarray(bo, np.float32)
    results = run_cores(shard_inputs(x, Wq, Wk, Wv, Wo))
    out = np.empty((B, S, D), np.float32)
    for b in range(B):
        out[b] = results[4 * b]["out"]
        for kv in range(1, HKV):
            out[b] += results[4 * b + kv]["out"]
        out[b] += bo
    return out


# revision 3
# speedup vs baseline: 1.4764x; 1.2055x over previous
"""GQA attention (B=2, S=2048, D=2048, Hq=16, Hkv=4, hd=128) on 8 TRN2 cores, v2.

Sharding: core c = b*4 + kv handles batch b and kv-head kv (with its 4 query
heads). Each core computes its partial output (A_heads @ Wo_slice); the host
sums the 4 partials per batch and adds the bias.

v2 vs v1: everything in bf16 (PE rate is the same 1 row/cycle as fp32r, but
DMA/SBUF halve and DVE gets 2x/4x 16-bit modes), x is loaded once (Q^T for the
whole sequence is kept in SBUF), and the PE instruction stream is packed so the
tensor engine never waits on the activation engine:
  pass A: Q^T/K^T for all blocks ([hd, s] layout), V^T -> PE-transpose -> V.
          Q for the last block is deferred into pass B as PE filler work.
  pass B: per 512-column block ib, per head: 16 j-steps of [ST matmul, paired
          1024-wide exp on ACT, PV matmul]; one filler matmul per step keeps
          PE ahead of ACT (fillers: deferred Q-proj for ib==0, out-projection
          of block ib-1 otherwise). Softmax denominators: bf16 tree adds split
          across DVE/Pool, partition_all_reduce on Pool, reciprocal+normalize
          on DVE -- nothing in the PE stream stalls on them.
"""
import sys

sys.path.insert(0, "/opt/trn_rl_repo")
import numpy as np

B, S, D = 2, 2048, 2048
HQ, HKV, HD = 16, 4, 128
G = HQ // HKV
SCALE = HD ** -0.5
P = 128
NB = 512
DC = D // P     # 16 contraction chunks
SB = S // NB    # 4 seq blocks of 512
ST = S // P     # 16 seq tiles of 128
DEPTH = 4       # PV trails ST by this many j-steps

_CACHE = {}


def _build(reps=(1, 1, 1)):
    from contextlib import ExitStack, nullcontext

    import concourse.bacc as bacc
    import concourse.bass_isa as bass_isa
    import concourse.mybir as mybir
    import concourse.tile as tile
    from concourse.masks import make_identity

    F32 = mybir.dt.float32
    BF16 = mybir.dt.bfloat16
    Exp = mybir.ActivationFunctionType.Exp
    MULT = mybir.AluOpType.mult

    nc = bacc.Bacc("TRN2", target_bir_lowering=False, debug=False)
    xT = nc.dram_tensor("xT", [D, S], BF16, kind="ExternalInput").ap()
    wq = nc.dram_tensor("wq", [D, G * HD], BF16, kind="ExternalInput").ap()
    wk = nc.dram_tensor("wk", [D, HD], BF16, kind="ExternalInput").ap()
    wv = nc.dram_tensor("wv", [D, HD], BF16, kind="ExternalInput").ap()
    wo = nc.dram_tensor("wo", [G * HD, D], BF16, kind="ExternalInput").ap()
    out = nc.dram_tensor("out", [S, D], F32, kind="ExternalOutput").ap()

    r1, r2, _ = reps

    with tile.TileContext(nc) as tc, ExitStack() as stk:
        persist = stk.enter_context(tc.tile_pool(name="persist", bufs=1))
        kt_sb = persist.tile([P, S], BF16)          # K^T [hd, j]
        v_sb = persist.tile([P, ST, HD], BF16)      # V   [j, (jt, hd)]
        qt_sb = persist.tile([P, G, S], BF16)       # Q^T [hd, (h, i)]
        xt3 = persist.tile([P, DC, NB], BF16)       # x^T cols of last block
        wq_sb = persist.tile([P, DC, G * HD], BF16)
        wk_sb = persist.tile([P, DC, HD], BF16)
        wv_sb = persist.tile([P, DC, HD], BF16)
        wo_sb = persist.tile([P, G, D], BF16)
        ident = persist.tile([P, P], BF16)
        ones = persist.tile([P, P], BF16)
        make_identity(nc, ident)
        nc.gpsimd.memset(ones, 1.0)
        for c in range(DC):
            nc.sync.dma_start(out=wq_sb[:, c, :], in_=wq[c * P:(c + 1) * P, :])
            nc.sync.dma_start(out=wk_sb[:, c, :], in_=wk[c * P:(c + 1) * P, :])
            nc.sync.dma_start(out=wv_sb[:, c, :], in_=wv[c * P:(c + 1) * P, :])
        for h in range(G):
            nc.sync.dma_start(out=wo_sb[:, h, :], in_=wo[h * P:(h + 1) * P, :])

        def _loop(r):
            return tc.For_i(0, r, 1) if r > 1 else nullcontext()

        # ---- pass A: K^T, V for all blocks; Q^T for blocks 0..SB-2 ----
        with ExitStack() as pas:
            xta_pool = pas.enter_context(tc.tile_pool(name="xta", bufs=2))
            vt_pool = pas.enter_context(tc.tile_pool(name="vt", bufs=2))
            ps_a = pas.enter_context(tc.tile_pool(name="ps_a", bufs=2, space="PSUM"))

            with _loop(r1):
              for xb in range(SB):
                cols = slice(xb * NB, (xb + 1) * NB)
                xt = xt3 if xb == SB - 1 else xta_pool.tile([P, DC, NB], BF16, name="xt")
                for c in range(DC):
                    nc.sync.dma_start(out=xt[:, c, :], in_=xT[c * P:(c + 1) * P, cols])
                pk = ps_a.tile([P, NB], F32, name="pkv")
                for c in range(DC):
                    nc.tensor.matmul(pk, wk_sb[:, c, :], xt[:, c, :],
                                     start=(c == 0), stop=(c == DC - 1))
                nc.vector.tensor_copy(out=kt_sb[:, cols], in_=pk)
                pv = ps_a.tile([P, NB], F32, name="pkv")
                for c in range(DC):
                    nc.tensor.matmul(pv, wv_sb[:, c, :], xt[:, c, :],
                                     start=(c == 0), stop=(c == DC - 1))
                vt = vt_pool.tile([P, NB], BF16, name="vt")
                nc.vector.tensor_copy(out=vt, in_=pv)
                # Q chains keep PE busy while the vt copy completes
                if xb != SB - 1:
                    for h in range(G):
                        pq = ps_a.tile([P, NB], F32, name="pq")
                        for c in range(DC):
                            nc.tensor.matmul(pq, wq_sb[:, c, h * HD:(h + 1) * HD],
                                             xt[:, c, :], start=(c == 0), stop=(c == DC - 1))
                        nc.vector.tensor_copy(out=qt_sb[:, h, cols], in_=pq)
                for k in range(NB // P):
                    pt = ps_a.tile([P, P], BF16, name="pt")
                    nc.tensor.transpose(pt, vt[:, k * P:(k + 1) * P], ident)
                    nc.vector.tensor_copy(out=v_sb[:, xb * (NB // P) + k, :], in_=pt)

        # ---- pass B: attention + out-projection, PE stream fully packed ----
        with ExitStack() as pbs:
            ex_pool = pbs.enter_context(tc.tile_pool(name="ex", bufs=2))
            ot_pool = pbs.enter_context(tc.tile_pool(name="otp", bufs=2))
            dn_pool = pbs.enter_context(tc.tile_pool(name="dn", bufs=2))
            st_pool = pbs.enter_context(tc.tile_pool(name="st", bufs=4))
            ps_st = pbs.enter_context(tc.tile_pool(name="ps_st", bufs=2, space="PSUM"))
            ps_po = pbs.enter_context(tc.tile_pool(name="ps_po", bufs=1, space="PSUM"))
            ps_aux = pbs.enter_context(tc.tile_pool(name="ps_aux", bufs=2, space="PSUM"))
            ps_pd = pbs.enter_context(tc.tile_pool(name="ps_pd", bufs=1, space="PSUM"))

            with _loop(r2):
                ot_tiles = {}
                aux_state = {}

                def q3_fillers():
                    # deferred Q projection for the last block: 64 matmuls
                    fillers = []
                    icols3 = slice((SB - 1) * NB, SB * NB)
                    for h in range(G):
                        for c in range(DC):
                            def go(h=h, c=c):
                                if c == 0:
                                    aux_state["q"] = ps_aux.tile([P, NB], F32, name="aux")
                                pq = aux_state["q"]
                                nc.tensor.matmul(pq, wq_sb[:, c, h * HD:(h + 1) * HD],
                                                 xt3[:, c, :], start=(c == 0), stop=(c == DC - 1))
                                if c == DC - 1:
                                    nc.vector.tensor_copy(out=qt_sb[:, h, icols3], in_=pq)
                            fillers.append(go)
                    return fillers

                def outproj_fillers(ib):
                    # 16 (t, nb) groups x 4 h-matmuls; two groups in flight so
                    # each group's h3 (which waits on the newest normalize) sits
                    # late in the queue
                    fillers = []
                    ot = ot_tiles[ib]
                    groups = [(t, nb) for t in range(4) for nb in range(D // NB)]
                    for p0 in range(0, 16, 2):
                        pair = groups[p0:p0 + 2]
                        for h in range(G):
                            for t, nb in pair:
                                def go(t=t, nb=nb, h=h, ib=ib, ot=ot):
                                    key = ("o", t, nb)
                                    if h == 0:
                                        aux_state[key] = ps_aux.tile([P, NB], F32, name="aux")
                                    pso = aux_state[key]
                                    nc.tensor.matmul(pso, ot[:, h, t * P:(t + 1) * P],
                                                     wo_sb[:, h, nb * NB:(nb + 1) * NB],
                                                     start=(h == 0), stop=(h == G - 1))
                                    if h == G - 1:
                                        so = st_pool.tile([P, NB], F32, name="so")
                                        # drain on DVE only: ACT must stay a
                                        # pure exp pipeline, and gpsimd cannot
                                        # read PSUM
                                        nc.vector.tensor_copy(out=so, in_=pso)
                                        it = 4 * ib + t
                                        nc.sync.dma_start(
                                            out=out[it * P:(it + 1) * P, nb * NB:(nb + 1) * NB],
                                            in_=so)
                                fillers.append(go)
                    return fillers

                pending_tail = [None]

                def head_tail(ib, h, dsum, oun):
                    # denominator broadcast-sum via PE ones-matmul (replicates
                    # the partition sum to all 128 partitions in fp32), then
                    # reciprocal on DVE + normalize on Pool (all-SBUF).
                    # Scheduled at step 8 of the NEXT head's j-loop so the PE
                    # never waits on the add tree.
                    def go():
                        pd = ps_pd.tile([P, NB], F32, name="pd")
                        nc.tensor.matmul(pd, ones, dsum, start=True, stop=True)
                        rec = dn_pool.tile([P, NB], F32, name="rec")
                        nc.vector.reciprocal(out=rec, in_=pd)
                        nc.gpsimd.tensor_tensor(out=ot_tiles[ib][:, h, :], in0=oun,
                                                in1=rec, op=MULT)
                    return go

                # one global filler queue: (min_slot, closure). Fillers whose
                # block ends before they are consumed spill into the next
                # block's leading slots, so the PE stream never has a hole
                # (holes drop the tensor engine out of its high p-state).
                filler_q = []
                cur_slot = [0]

                def pop_filler():
                    if filler_q and filler_q[0][0] <= cur_slot[0]:
                        filler_q.pop(0)[1]()

                for ib in range(SB):
                    icols = slice(ib * NB, (ib + 1) * NB)
                    ot_tiles[ib] = ot_pool.tile([P, G, NB], BF16, name="ot")
                    if ib == 0:
                        filler_q += [(0, f) for f in q3_fillers()]
                    else:
                        # out-projection h3 matmuls wait on the normalize that
                        # runs at step 12 of this block's first head loop
                        start = cur_slot[0] + 8
                        filler_q += [(start, f) for f in outproj_fillers(ib - 1)]
                    for h in range(G):
                        ex = ex_pool.tile([P, ST, NB], BF16, name="ex")
                        po = ps_po.tile([P, NB], F32, name="po")
                        pss = None
                        for j in range(ST + DEPTH):
                            if j < ST:
                                if j % 2 == 0:
                                    pss = ps_st.tile([P, 2, NB], F32, name="pss")
                                nc.tensor.matmul(pss[:, j % 2, :], kt_sb[:, j * P:(j + 1) * P],
                                                 qt_sb[:, h, icols], start=True, stop=True)
                                if j % 2 == 1:
                                    nc.scalar.activation(out=ex[:, j - 1:j + 1, :], in_=pss,
                                                         func=Exp, scale=SCALE)
                            jc = j - DEPTH
                            if jc >= 0:
                                nc.tensor.matmul(po, v_sb[:, jc, :], ex[:, jc, :],
                                                 start=(jc == 0), stop=(jc == ST - 1))
                            if j == 12 and pending_tail[0] is not None:
                                pending_tail[0]()
                                pending_tail[0] = None
                            if j < ST:
                                pop_filler()
                                cur_slot[0] += 1
                        # drain the PV accumulator unnormalized right away so
                        # its PSUM bank frees before the next head's PV chain
                        oun = dn_pool.tile([P, NB], BF16, name="oun")
                        nc.vector.tensor_copy(out=oun, in_=po)
                        # softmax denominator add tree: bf16, DVE half (can
                        # start mid-loop) + Pool half; final sums in bf16 feed
                        # the ones-matmul
                        t4a = dn_pool.tile([P, 4, NB], BF16, name="t4a")
                        nc.vector.tensor_add(out=t4a, in0=ex[:, 0:4, :], in1=ex[:, 4:8, :])
                        nc.vector.tensor_add(out=t4a[:, 0:2, :], in0=t4a[:, 0:2, :],
                                             in1=t4a[:, 2:4, :])
                        t4b = dn_pool.tile([P, 4, NB], BF16, name="t4b")
                        nc.gpsimd.tensor_add(out=t4b, in0=ex[:, 8:12, :], in1=ex[:, 12:16, :])
                        nc.gpsimd.tensor_add(out=t4b[:, 0:2, :], in0=t4b[:, 0:2, :],
                                             in1=t4b[:, 2:4, :])
                        dsa = dn_pool.tile([P, NB], BF16, name="dsa")
                        nc.vector.tensor_add(out=dsa, in0=t4a[:, 0, :], in1=t4a[:, 1, :])
                        dsum = dn_pool.tile([P, NB], BF16, name="dsum")
                        nc.gpsimd.tensor_add(out=dsum, in0=t4b[:, 0, :], in1=t4b[:, 1, :])
                        nc.gpsimd.tensor_add(out=dsum, in0=dsum, in1=dsa)
                        pending_tail[0] = head_tail(ib, h, dsum, oun)
                # trailing: leftover fillers, last head's tail, then the last
                # block's out-projection
                while filler_q:
                    filler_q.pop(0)[1]()
                pending_tail[0]()
                pending_tail[0] = None
                for _, f in [(0, f) for f in outproj_fillers(SB - 1)]:
                    f()

    nc.compile()
    return nc


def _get_nc():
    if "nc" not in _CACHE:
        _CACHE["nc"] = _build()
    return _CACHE["nc"]


def timed_runner(reps):
    nc = _build(reps)
    return make_runner(nc)


def make_runner(nc, n_cores=8):
    """Persistent jitted SPMD runner (mirrors bass2jax.run_bass_via_pjrt's
    multi-core path, without donation so the executable can be re-invoked on
    device-resident inputs for timing)."""
    import jax
    from jax.experimental.shard_map import shard_map
    from jax.sharding import Mesh, PartitionSpec

    import concourse.mybir as mybir
    from concourse import bass2jax

    bass2jax.install_neuronx_cc_hook()
    partition_name = nc.partition_id_tensor.name if nc.partition_id_tensor else None
    in_names, out_names, out_avals, zero_shapes = [], [], [], []
    for alloc in nc.m.functions[0].allocations:
        if not isinstance(alloc, mybir.MemoryLocationSet):
            continue
        name = alloc.memorylocations[0].name
        if alloc.kind == "ExternalInput":
            if name != partition_name:
                in_names.append(name)
        elif alloc.kind == "ExternalOutput":
            out_names.append(name)
            shape = tuple(alloc.tensor_shape)
            dtype = mybir.dt.np(alloc.dtype)
            out_avals.append(jax.core.ShapedArray(shape, dtype))
            zero_shapes.append((shape, dtype))
    n_params = len(in_names)
    all_in_names = tuple(in_names + out_names)
    if partition_name is not None:
        all_in_names = all_in_names + (partition_name,)

    def _body(*args):
        operands = list(args)
        if partition_name is not None:
            operands.append(bass2jax.partition_id_tensor())
        outs = bass2jax._bass_exec_p.bind(
            *operands,
            out_avals=tuple(out_avals),
            in_names=all_in_names,
            out_names=tuple(out_names),
            lowering_input_output_aliases=(),
            sim_require_finite=True,
            sim_require_nnan=True,
            nc=nc,
        )
        return tuple(outs)

    devices = jax.devices()[:n_cores]
    mesh = Mesh(np.asarray(devices), ("core",))
    n_outs = len(out_names)
    fn = jax.jit(
        shard_map(_body, mesh=mesh,
                  in_specs=(PartitionSpec("core"),) * (n_params + n_outs),
                  out_specs=(PartitionSpec("core"),) * n_outs,
                  check_rep=False),
        keep_unused=True,
    )
    return fn, in_names, out_names, zero_shapes, mesh


def _get_runner():
    if "runner" not in _CACHE:
        _CACHE["runner"] = make_runner(_get_nc())
    return _CACHE["runner"]


def run_cores(in_maps):
    """Run the 8-core SPMD program; returns list of per-core {name: array}."""
    import jax

    fn, in_names, out_names, zero_shapes, mesh = _get_runner()
    n = len(in_maps)
    concat_in = [np.concatenate([np.asarray(in_maps[c][nm]) for c in range(n)], axis=0)
                 for nm in in_names]
    concat_zero = [np.zeros((n * s[0], *s[1:]), dt) for s, dt in zero_shapes]
    outs = fn(*concat_in, *concat_zero)
    outs = [np.asarray(o) for o in outs]
    return [
        {nm: outs[i].reshape(n, *zero_shapes[i][0])[c] for i, nm in enumerate(out_names)}
        for c in range(n)
    ]


def shard_inputs(x, Wq, Wk, Wv, Wo):
    import ml_dtypes

    bf16 = ml_dtypes.bfloat16
    in_maps = []
    for b in range(B):
        xTb = np.ascontiguousarray(x[b].T).astype(bf16)
        for kv in range(HKV):
            in_maps.append({
                "xT": xTb,
                "wq": np.ascontiguousarray(Wq[:, kv * G * HD:(kv + 1) * G * HD]).astype(bf16),
                "wk": np.ascontiguousarray(Wk[:, kv * HD:(kv + 1) * HD]).astype(bf16),
                "wv": np.ascontiguousarray(Wv[:, kv * HD:(kv + 1) * HD]).astype(bf16),
                "wo": np.ascontiguousarray(Wo[kv * G * HD:(kv + 1) * G * HD, :]).astype(bf16),
            })
    return in_maps


def kernel(x, Wq, Wk, Wv, Wo, bo):
    x = np.asarray(x, np.float32)
    Wq = np.asarray(Wq, np.float32)
    Wk = np.asarray(Wk, np.float32)
    Wv = np.asarray(Wv, np.float32)
    Wo = np.asarray(Wo, np.float32)
    bo = np.asarray(bo, np.float32)
    results = run_cores(shard_inputs(x, Wq, Wk, Wv, Wo))
    out = np.empty((B, S, D), np.float32)
    for b in range(B):
        out[b] = results[4 * b]["out"]
        for kv in range(1, HKV):
            out[b] += results[4 * b + kv]["out"]
        out[b] += bo
    return out
